# revision 1
# baseline (speedup 1.0000x reference)
"""Trainium2 Bass kernel for nn_NaturalGradientDescentVelNet.

Data-parallel over 8 NeuronCores: each core processes N/8 = 16384 points.
Per core, points are processed in 4 "super-tiles" of 8x512-point tiles.

Per tile (H-phase, hidden-dim-on-partitions layout [H, 512]):
  block A: taskmap forward (tanh MLP + elu MLP) + Jacobian tangent
           propagation (2 tangents, negated-sign trick), all ACT funcs from
           the exp_and_others table set.
  block B: softplus via ln(1+e^q3)  (natural_log_exp set -- one table
           switch per super-tile).
  block C: y = (1+s)*y1 - origin, vv net (PReLU MLP), vs net (leaky MLP),
           PE-transposes of 19 packed per-point scalars into a
           points-on-partitions B-layout tile.
  block D (per super-tile, B-layout [128, 32 groups x 19]): all per-point
           math -- sigmoid, softplus consumers, yd projection, normalize
           (ln/exp rsqrt + Newton), 2x2 adjugate inverse, vel scalar exp.

Matmul dtype per net (PE cost: f32 = 4 cyc/row, f32r = 1 cyc/row):
  tm1 fwd f32; tm2 fwd f32r; tangents f32r; vv_w1 f32r; vv_w2/w3 f32;
  vs f32r.  Host-simulated end-to-end scale-relative error ~9e-4.
"""
import numpy as np
import concourse.bass as bass
import concourse.mybir as mybir
import concourse.tile as tile
from concourse.bass_utils import run_bass_kernel_spmd

F = mybir.ActivationFunctionType
DT = mybir.dt
AL = mybir.AluOpType

N_CORES = 8
N_TOTAL = 131072
N_CORE = N_TOTAL // N_CORES       # 16384
TB = 512                          # points per tile
N_TILES = N_CORE // TB            # 32
ST_TILES = 8                      # tiles per super-tile
N_ST = N_TILES // ST_TILES        # 4
NG = ST_TILES * 4                 # 32 groups of 128 points per super-tile
NROW = 19                         # packed per-point scalars

# pack row offsets
R_X, R_Y, R_Y1, R_Q3, R_DOTY, R_LV = 0, 2, 4, 6, 8, 10
R_DY10, R_DY11, R_P30, R_P31 = 11, 13, 15, 17

DT_TM1 = "f32"    # tm1 forward
DT_TM2 = "f32"    # tm2 forward
DT_TG = "f32"     # tangents
DT_VV1 = "f32"    # vv layer 1
DT_VV23 = "f32"   # vv layers 2,3
DT_VS = "f32"     # vs net


def _f32r(dt_key):
    return DT.float32r if dt_key == "f32r" else DT.float32


def fix_sync_waits(nc, limit=1):
    """Hoist excess sem waits onto same-engine NoOps (walrus codegen limit)."""
    for fn in nc.m.functions:
        for bb in fn.blocks:
            insts = bb.instructions
            idx = 0
            while idx < len(insts):
                inst = insts[idx]
                si = inst.sync_info
                if si is not None and len(si.on_wait) > limit:
                    extra = list(si.on_wait[limit:])
                    del si.on_wait[limit:]
                    for k, w in enumerate(extra):
                        nop = mybir.InstNoOp(
                            name=f"{inst.name}-wnop{k}",
                            engine=inst.engine,
                            sync_info=mybir.SyncInfo(on_wait=[w], on_update=[]),
                        )
                        try:
                            nc.register_instruction(nop, overwrite=True)
                        except Exception:
                            pass
                        insts.insert(idx, nop)
                        idx += 1
                idx += 1


def _host_prep(inp):
    """Derived host-side constants. Returns dict of extra DRAM arrays + alphas."""
    f = {k: np.asarray(v, np.float32) for k, v in inp.items()}
    d = {}
    col = lambda a: np.ascontiguousarray(np.asarray(a, np.float32).reshape(-1, 1))
    # biases as [H,1]
    d["b1"] = col(f["tm1_b1"]); d["b2"] = col(f["tm1_b2"]); d["b3"] = col(f["tm1_b3"])
    c1 = f["tm2_b1"]
    c2p = f["tm2_b2"] - f["tm2_w2"].sum(0)
    c3p = f["tm2_b3"] - f["tm2_w3"].sum(0)
    d["c1"] = col(c1); d["nc1"] = col(-c1)
    d["c2p"] = col(c2p); d["nc2p"] = col(-c2p)
    d["c3p"] = col(c3p)
    d["vb1"] = col(f["vv_b1"]); d["vb2"] = col(f["vv_b2"]); d["vb3"] = col(f["vv_b3"])
    d["sb1"] = col(f["vs_b1"]); d["sb2"] = col(f["vs_b2"]); d["sb3"] = col(f["vs_b3"])
    # tangent seed columns (dh1'_j = u1*W1[j] - W1[j] = -(1-h1^2)W1[j])
    d["w1p0"] = col(f["tm1_w1"][0]); d["w1n0"] = col(-f["tm1_w1"][0])
    d["w1p1"] = col(f["tm1_w1"][1]); d["w1n1"] = col(-f["tm1_w1"][1])
    d["e0"] = col(np.array([1.0, 0.0])); d["e1c"] = col(np.array([0.0, 1.0]))
    d["eye"] = np.eye(NROW, dtype=np.float32)
    # origin_y = taskmap(0) in float64
    g = {k: np.asarray(v, np.float64) for k, v in inp.items()}
    z = np.zeros((1, 2))
    h = np.tanh(z @ g["tm1_w1"] + g["tm1_b1"])
    h = np.tanh(h @ g["tm1_w2"] + g["tm1_b2"])
    y1 = h @ g["tm1_w3"] + g["tm1_b3"] + z
    q = y1 @ g["tm2_w1"] + g["tm2_b1"]; gq = np.where(q > 0, q, np.expm1(q))
    q = gq @ g["tm2_w2"] + g["tm2_b2"]; gq = np.where(q > 0, q, np.expm1(q))
    q = gq @ g["tm2_w3"] + g["tm2_b3"]
    s = np.log1p(np.exp(-np.abs(q))) + np.maximum(q, 0)
    origin = (s * y1 + y1)[0]
    d["oy"] = col(origin)
    alphas = (float(f["vv_a1"][0]), float(f["vv_a2"][0]))
    # weights passed through as-is
    for k in ["tm1_w1", "tm1_w2", "tm1_w3", "tm2_w1", "tm2_w2", "tm2_w3",
              "vv_w1", "vv_w2", "vv_w3", "vs_w1", "vs_w2", "vs_w3"]:
        d[k] = f[k]
    return d, alphas


def build_program(alphas, debug=False, modes=None):
    """Build the SPMD Bass program (same for all cores)."""
    a1, a2 = alphas
    m = {"tm1": DT_TM1, "tm2": DT_TM2, "tg": DT_TG, "vv1": DT_VV1,
         "vv23": DT_VV23, "vs": DT_VS}
    if modes:
        m.update(modes)
    assert m["vv23"] == "f32", "f32r vv23 chunks not wired"
    nc = bass.Bass()
    dbg = {}
    def dbg_out(name, shape):
        if name not in dbg:
            dbg[name] = nc.declare_dram_parameter("dbg_" + name, list(shape), DT.float32, isOutput=True)
        return dbg[name]

    x_ext = nc.declare_dram_parameter("x", [N_CORE, 2], DT.float32, isOutput=False)
    out_ext = nc.declare_dram_parameter("xd", [N_CORE, 2], DT.float32, isOutput=True)

    shapes = {
        "tm1_w1": [2, 100], "tm1_w2": [100, 100], "tm1_w3": [100, 2],
        "tm2_w1": [2, 100], "tm2_w2": [100, 100], "tm2_w3": [100, 2],
        "vv_w1": [2, 300], "vv_w2": [300, 300], "vv_w3": [300, 2],
        "vs_w1": [2, 100], "vs_w2": [100, 100], "vs_w3": [100, 1],
        "b1": [100, 1], "b2": [100, 1], "b3": [2, 1],
        "c1": [100, 1], "nc1": [100, 1], "c2p": [100, 1], "nc2p": [100, 1],
        "c3p": [2, 1],
        "vb1": [300, 1], "vb2": [300, 1], "vb3": [2, 1],
        "sb1": [100, 1], "sb2": [100, 1], "sb3": [1, 1],
        "w1p0": [100, 1], "w1n0": [100, 1], "w1p1": [100, 1], "w1n1": [100, 1],
        "e0": [2, 1], "e1c": [2, 1], "oy": [2, 1], "eye": [NROW, NROW],
    }
    ext = {k: nc.declare_dram_parameter(k, v, DT.float32, isOutput=False)
           for k, v in shapes.items()}

    XR = x_ext.rearrange("(t n) d -> t d n", n=TB)             # [32, 2, 512]
    OUTR = out_ext.rearrange("(s g p) d -> s p g d", g=NG, p=128)  # [4, 128, 32, 2]

    VCH = [(0, 128), (128, 128), (256, 44)]  # K/M chunks of 300

    from contextlib import ExitStack
    with tile.TileContext(nc) as tc, ExitStack() as es:
        cpool = es.enter_context(tc.tile_pool(name="const", bufs=1))
        pool = es.enter_context(tc.tile_pool(name="work", bufs=1))
        pp = es.enter_context(tc.tile_pool(name="ps", bufs=1, space="PSUM"))

        # ---- constants into SBUF (chunk-only tensors excluded) ----
        CHUNK_ONLY = {"vv_w2", "vv_w3", "vb1", "vb2"}
        ct = {}
        for k, shp in shapes.items():
            if k in CHUNK_ONLY:
                continue
            t = cpool.tile(shp, DT.float32, tag="c_" + k)
            nc.sync.dma_start(t[:], ext[k][:])
            ct[k] = t
        # chunked vv weights / biases
        vv_w2f = []
        vv_w3f = []
        a_vb1, a_vb2 = [], []
        for (o, sz) in VCH:
            t = cpool.tile([sz, 300], DT.float32, tag=f"c_vvw2_{o}")
            nc.sync.dma_start(t[:], ext["vv_w2"][o:o + sz, :])
            vv_w2f.append(t)
            t = cpool.tile([sz, 2], DT.float32, tag=f"c_vvw3_{o}")
            nc.sync.dma_start(t[:], ext["vv_w3"][o:o + sz, :])
            vv_w3f.append(t)
            t = cpool.tile([sz, 1], DT.float32, tag=f"c_vb1_{o}")
            nc.sync.dma_start(t[:], ext["vb1"][o:o + sz, :])
            a_vb1.append(t)
            t = cpool.tile([sz, 1], DT.float32, tag=f"c_vb2_{o}")
            nc.sync.dma_start(t[:], ext["vb2"][o:o + sz, :])
            a_vb2.append(t)

        # f32r-rounded weight copies (producer must round for f32r matmuls)
        def r_copy(name, src):
            t = cpool.tile(list(src.shape), DT.float32r, tag="cr_" + name,
                           name="cr_" + name)
            nc.vector.tensor_copy(t[:], src[:])
            return t
        rcache = {}
        def wsel(name, mode):
            if mode == "f32":
                return ct[name]
            if name not in rcache:
                rcache[name] = r_copy(name, ct[name])
            return rcache[name]
        w_tm1w2_tg = wsel("tm1_w2", m["tg"])
        w_tm1w3_tg = wsel("tm1_w3", m["tg"])
        w_tm2w1_f = wsel("tm2_w1", m["tm2"])
        w_tm2w2_f = wsel("tm2_w2", m["tm2"])
        w_tm2w3_f = wsel("tm2_w3", m["tm2"])
        w_tm2w1_tg = wsel("tm2_w1", m["tg"])
        w_tm2w2_tg = wsel("tm2_w2", m["tg"])
        w_tm2w3_tg = wsel("tm2_w3", m["tg"])
        w_vv1 = wsel("vv_w1", m["vv1"])
        w_vs1 = wsel("vs_w1", m["vs"])
        w_vs2 = wsel("vs_w2", m["vs"])
        w_vs3 = wsel("vs_w3", m["vs"])
        DTG = _f32r(m["tg"]); DTM2 = _f32r(m["tm2"])
        DVV1 = _f32r(m["vv1"]); DVS = _f32r(m["vs"])

        MM = nc.tensor.matmul
        ACT = nc.scalar.activation
        V = nc.vector

        def h_block_A(t, pack, eqp):
            """taskmap fwd + tangents for tile t. Writes pack rows and
            eqp = 1 + exp(q3). Returns f32r dy1 tiles."""
            nc.sync.dma_start(pack[R_X:R_X + 2, :], XR[t])
            # tm1 forward (f32)
            ps = pp.tile([128, TB], DT.float32, tag="ps", bufs=6)
            MM(ps[0:100, :], ct["tm1_w1"][:], pack[R_X:R_X + 2, :], start=True, stop=True)
            h1 = pool.tile([100, TB], DT.float32, tag="h1", bufs=2)
            ACT(h1[:], ps[0:100, :], F.Tanh, bias=ct["b1"][:])
            u1 = pool.tile([100, TB], DT.float32, tag="u1", bufs=1)
            ACT(u1[:], h1[:], F.Square)
            ps2 = pp.tile([128, TB], DT.float32, tag="ps", bufs=6)
            MM(ps2[0:100, :], ct["tm1_w2"][:], h1[:], start=True, stop=True)
            h2 = pool.tile([100, TB], DT.float32, tag="h2", bufs=2)
            ACT(h2[:], ps2[0:100, :], F.Tanh, bias=ct["b2"][:])
            u2 = pool.tile([100, TB], DT.float32, tag="u2", bufs=1)
            ACT(u2[:], h2[:], F.Square)
            ps3 = pp.tile([128, TB], DT.float32, tag="ps", bufs=6)
            MM(ps3[0:2, :], ct["tm1_w3"][:], h2[:], start=True, stop=True)
            y1t = pool.tile([2, TB], DT.float32, tag=f"y1t{t % 8}", bufs=2)
            V.tensor_scalar(y1t[:], ps3[0:2, :], ct["b3"][:], None, AL.add)
            V.tensor_add(y1t[:], y1t[:], pack[R_X:R_X + 2, :])
            nc.sync.dma_start(pack[R_Y1:R_Y1 + 2, :], y1t[:])
            if m["tm2"] == "f32r":
                y1r = pool.tile([2, TB], DT.float32r, tag="y1r", bufs=1)
                V.tensor_copy(y1r[:], y1t[:])
            else:
                y1r = y1t

            # tm1 tangents (negated): dh1'_j = u1*w1p_j - w1p_j
            dy1r = []
            dh2r = []
            for j, (wp, wn) in enumerate([("w1p0", "w1n0"), ("w1p1", "w1n1")]):
                dh1 = pool.tile([100, TB], DTG, tag=f"dh1_{j}", bufs=1)
                V.tensor_scalar(dh1[:], u1[:], ct[wp][:], ct[wn][:], AL.mult, AL.add)
                psd = pp.tile([128, TB], DT.float32, tag="ps", bufs=6)
                MM(psd[0:100, :], w_tm1w2_tg[:], dh1[:], start=True, stop=True)
                dh2 = pool.tile([100, TB], DTG, tag=f"dh2_{j}", bufs=1)
                # dh2' = (u2-1)*psd = (1-h2^2)*(true tangent)
                V.scalar_tensor_tensor(dh2[:], u2[:], 1.0, psd[0:100, :], AL.subtract, AL.mult)
                dh2r.append(dh2)
            for j, (dh2, ec, rowo) in enumerate([(dh2r[0], "e0", R_DY10), (dh2r[1], "e1c", R_DY11)]):
                psd = pp.tile([128, TB], DT.float32, tag="ps", bufs=6)
                MM(psd[0:2, :], w_tm1w3_tg[:], dh2[:], start=True, stop=True)
                # dh2' double-negated back to true sign, so dy1 = psd + e_j
                dyt = pool.tile([2, TB], DT.float32, tag=f"dyt_{j}", bufs=1)
                V.tensor_scalar(dyt[:], psd[0:2, :], ct[ec][:], None, AL.add)
                nc.sync.dma_start(pack[rowo:rowo + 2, :], dyt[:])
                if m["tg"] == "f32r":
                    dr = pool.tile([2, TB], DT.float32r, tag=f"dy1r_{j}", bufs=1)
                    V.tensor_copy(dr[:], dyt[:])
                else:
                    dr = dyt
                dy1r.append(dr)

            # tm2 forward (f32r) with elu
            def elu_layer(rhs, cb, ncb, tagp):
                psq = pp.tile([128, TB], DT.float32, tag="ps", bufs=6)
                wq = w_tm2w1_f if tagp == "1" else w_tm2w2_f
                MM(psq[0:100, :], wq[:], rhs[:], start=True, stop=True)
                rn = pool.tile([100, TB], DT.float32, tag="rn" + tagp, bufs=1)
                ACT(rn[:], psq[0:100, :], F.Relu, bias=ct[ncb][:], scale=-1.0)
                e = pool.tile([100, TB], DT.float32, tag="e" + tagp, bufs=1)
                ACT(e[:], rn[:], F.Exp, scale=-1.0)
                gh = pool.tile([100, TB], DT.float32, tag="gh" + tagp, bufs=1)
                ACT(gh[:], psq[0:100, :], F.Relu, bias=ct[cb][:])
                gu = pool.tile([100, TB], DTM2, tag="gu" + tagp, bufs=1)
                V.tensor_add(gu[:], gh[:], e[:])
                return e, gu
            e1t, g1u = elu_layer(y1r, "c1", "nc1", "1")
            e2t, g2u = elu_layer(g1u, "c2p", "nc2p", "2")
            psq3 = pp.tile([128, TB], DT.float32, tag="ps", bufs=6)
            MM(psq3[0:2, :], w_tm2w3_f[:], g2u[:], start=True, stop=True)
            q3t = pool.tile([2, TB], DT.float32, tag="q3t", bufs=1)
            V.tensor_scalar(q3t[:], psq3[0:2, :], ct["c3p"][:], None, AL.add)
            nc.sync.dma_start(pack[R_Q3:R_Q3 + 2, :], q3t[:])
            # eqp = exp(q3) + 1  (ln input for softplus)
            ACT(eqp[:], psq3[0:2, :], F.Exp, bias=ct["c3p"][:])
            V.tensor_scalar(eqp[:], eqp[:], 1.0, None, AL.add)

            # tm2 tangents (negated stream)
            for j, (dr, rowo) in enumerate([(dy1r[0], R_P30), (dy1r[1], R_P31)]):
                # note: matmul wants f32r rhs; dr is true-sign f32r
                psg = pp.tile([128, TB], DT.float32, tag="ps", bufs=6)
                MM(psg[0:100, :], w_tm2w1_tg[:], dr[:], start=True, stop=True)
                dg1 = pool.tile([100, TB], DTG, tag=f"dg1_{j}", bufs=1)
                V.tensor_mul(dg1[:], e1t[:], psg[0:100, :])
                psg2 = pp.tile([128, TB], DT.float32, tag="ps", bufs=6)
                MM(psg2[0:100, :], w_tm2w2_tg[:], dg1[:], start=True, stop=True)
                dg2 = pool.tile([100, TB], DTG, tag=f"dg2_{j}", bufs=1)
                V.tensor_mul(dg2[:], e2t[:], psg2[0:100, :])
                psg3 = pp.tile([128, TB], DT.float32, tag="ps", bufs=6)
                MM(psg3[0:2, :], w_tm2w3_tg[:], dg2[:], start=True, stop=True)
                p3t = pool.tile([2, TB], DT.float32, tag=f"p3t_{j}", bufs=1)
                V.tensor_copy(p3t[:], psg3[0:2, :])
                nc.sync.dma_start(pack[rowo:rowo + 2, :], p3t[:])
            if debug and t == 0:
                nc.sync.dma_start(dbg_out("h1", [100, TB])[:], h1[:])
                nc.sync.dma_start(dbg_out("e1", [100, TB])[:], e1t[:])
                nc.sync.dma_start(dbg_out("g1u", [100, TB])[:], g1u[:].bitcast(DT.float32))
                nc.sync.dma_start(dbg_out("y1t", [2, TB])[:], y1t[:])
                nc.sync.dma_start(dbg_out("q3t", [2, TB])[:], q3t[:])
            return y1t

        def h_block_C(t, tl, pack, s_t, y1t, btile):
            """y, vv net, vs net, transpose into btile cols for tile t."""
            # y = (s+1)*y1 - origin
            ypre = pool.tile([2, TB], DT.float32, tag="ypre", bufs=1)
            V.scalar_tensor_tensor(ypre[:], s_t[:], 1.0, y1t[:], AL.add, AL.mult)
            yt = pool.tile([2, TB], DT.float32, tag="yt", bufs=1)
            V.tensor_scalar(yt[:], ypre[:], ct["oy"][:], None, AL.subtract)
            nc.sync.dma_start(pack[R_Y:R_Y + 2, :], yt[:])
            if m["vv1"] == "f32r":
                y_r = pool.tile([2, TB], DT.float32r, tag="y_r", bufs=1)
                V.tensor_copy(y_r[:], yt[:])
            else:
                y_r = yt
            if m["vs"] == "f32r":
                x_r = pool.tile([2, TB], DT.float32r, tag="x_r", bufs=1)
                V.tensor_copy(x_r[:], pack[R_X:R_X + 2, :])
            else:
                x_r = None  # use pack slice directly

            # vv layer 1 (f32r) + prelu
            a1t = []
            for ci, (o, sz) in enumerate(VCH):
                psv = pp.tile([128, TB], DT.float32, tag="ps", bufs=6)
                MM(psv[0:sz, :], w_vv1[:, o:o + sz], y_r[:], start=True, stop=True)
                at = pool.tile([sz, TB], DT.float32, tag=f"a1_{ci}", bufs=1)
                ACT(at[:], psv[0:sz, :], F.Prelu, bias=a_vb1[ci][:], alpha=a1)
                a1t.append(at)
            # vv layer 2 (f32) + prelu
            a2t = []
            for ci, (o, sz) in enumerate(VCH):
                psv = pp.tile([128, TB], DT.float32, tag="ps", bufs=6)
                for ki, (ko, ksz) in enumerate(VCH):
                    MM(psv[0:sz, :], vv_w2f[ki][:, o:o + sz], a1t[ki][:],
                       start=(ki == 0), stop=(ki == 2))
                at = pool.tile([sz, TB], DT.float32, tag=f"a2_{ci}", bufs=1)
                ACT(at[:], psv[0:sz, :], F.Prelu, bias=a_vb2[ci][:], alpha=a2)
                a2t.append(at)
            # vv layer 3 (f32)
            psd = pp.tile([128, TB], DT.float32, tag="ps", bufs=6)
            for ki, (ko, ksz) in enumerate(VCH):
                MM(psd[0:2, :], vv_w3f[ki][:], a2t[ki][:], start=(ki == 0), stop=(ki == 2))
            dotyt = pool.tile([2, TB], DT.float32, tag="dotyt", bufs=1)
            V.tensor_scalar(dotyt[:], psd[0:2, :], ct["vb3"][:], None, AL.add)
            nc.sync.dma_start(pack[R_DOTY:R_DOTY + 2, :], dotyt[:])

            # vs net (f32r)
            psr = pp.tile([128, TB], DT.float32, tag="ps", bufs=6)
            xin = x_r[:] if x_r is not None else pack[R_X:R_X + 2, :]
            MM(psr[0:100, :], w_vs1[:], xin, start=True, stop=True)
            l1 = pool.tile([100, TB], DVS, tag="l1", bufs=1)
            ACT(l1[:], psr[0:100, :], F.Prelu, bias=ct["sb1"][:], alpha=0.01)
            psr2 = pp.tile([128, TB], DT.float32, tag="ps", bufs=6)
            MM(psr2[0:100, :], w_vs2[:], l1[:], start=True, stop=True)
            l2 = pool.tile([100, TB], DVS, tag="l2", bufs=1)
            ACT(l2[:], psr2[0:100, :], F.Prelu, bias=ct["sb2"][:], alpha=0.01)
            psr3 = pp.tile([128, TB], DT.float32, tag="ps", bufs=6)
            MM(psr3[0:1, :], w_vs3[:], l2[:], start=True, stop=True)
            lvt = pool.tile([1, TB], DT.float32, tag="lvt", bufs=1)
            V.tensor_scalar(lvt[:], psr3[0:1, :], ct["sb3"][:], None, AL.add)
            nc.sync.dma_start(pack[R_LV:R_LV + 1, :], lvt[:])
            if debug and t == 0:
                nc.sync.dma_start(dbg_out("s0", [2, TB])[:], s_t[:])
                nc.sync.dma_start(dbg_out("yt", [2, TB])[:], yt[:])
                nc.sync.dma_start(dbg_out("a1c0", [128, TB])[:], a1t[0][:])
                nc.sync.dma_start(dbg_out("l1", [100, TB])[:], l1[:].bitcast(DT.float32))
                nc.sync.dma_start(dbg_out("lvt", [1, TB])[:], lvt[:])
                nc.sync.dma_start(dbg_out("pack0", [NROW, TB])[:], pack[:])

            # transpose pack -> btile  (4 chunks of 128 points)
            psT = pp.tile([128, 4 * NROW], DT.float32, tag="psT", bufs=2)
            for u in range(4):
                nc.tensor.transpose(psT[:, u * NROW:(u + 1) * NROW],
                                    pack[:, u * 128:(u + 1) * 128], ct["eye"][:])
            V.tensor_copy(btile[:, tl * 4 * NROW:(tl + 1) * 4 * NROW], psT[:])

        def b_block_D(st, btile):
            """per-point math for one super-tile; writes output DMA."""
            BV = btile[:].rearrange("p (g r) -> p g r", r=NROW)
            sl = lambda k, w: BV[:, :, k:k + w]

            def W(tag, w, b=1):
                return pool.tile([128, NG * w], DT.float32, tag="bw_" + tag,
                                 bufs=b, name=f"bw_{tag}_{st}")
            def WV(t, w):
                return t[:].rearrange("p (g r) -> p g r", r=w)

            e_t = W("e", 2); e = WV(e_t, 2)
            ACT(e_t[:], sl(R_Q3, 2), F.Exp)
            lnin_t = W("lnin", 3); lnin = WV(lnin_t, 3)
            V.tensor_scalar(lnin[:, :, 0:2], e[:], 1.0, None, AL.add)
            # yd path
            p2_t = W("p2", 2); p2 = WV(p2_t, 2)
            V.tensor_mul(p2[:], sl(R_DOTY, 2), sl(R_Y, 2))
            ls_t = W("ls", 1); ls = WV(ls_t, 1)
            V.tensor_add(ls[:], p2[:, :, 0:1], p2[:, :, 1:2])
            V.tensor_mul(p2[:], sl(R_Y, 2), sl(R_Y, 2))
            vy_t = W("vy", 1); vy = WV(vy_t, 1)
            V.tensor_add(vy[:], p2[:, :, 0:1], p2[:, :, 1:2])
            rv_t = W("rv", 1); rv = WV(rv_t, 1)
            V.scalar_tensor_tensor(rv[:], vy[:], 1e-4, ls[:], AL.mult, AL.add)
            V.tensor_scalar(rv[:], rv[:], 0.0, None, AL.max)
            den_t = W("den", 1); den = WV(den_t, 1)
            V.tensor_scalar(den[:], vy[:], 1e-12, None, AL.add)
            V.reciprocal(den[:], den[:])
            V.tensor_mul(rv[:], rv[:], den[:])          # coef
            yd_t = W("yd", 2); yd = WV(yd_t, 2)
            for c in range(2):
                V.tensor_mul(yd[:, :, c:c + 1], rv[:], sl(R_Y + c, 1))
                V.tensor_sub(yd[:, :, c:c + 1], sl(R_DOTY + c, 1), yd[:, :, c:c + 1])
            V.tensor_mul(p2[:], yd[:], yd[:])
            V.tensor_add(lnin[:, :, 2:3], p2[:, :, 0:1], p2[:, :, 1:2])
            V.tensor_scalar(lnin[:, :, 2:3], lnin[:, :, 2:3], 1e-24, None, AL.max)
            ln_t = W("ln", 3); lnv = WV(ln_t, 3)
            ACT(ln_t[:], lnin_t[:], F.Ln)
            s_b = lnv[:, :, 0:2]
            rn_t = W("rn", 1); rn = WV(rn_t, 1)
            ACT(rn_t[:], lnv[:, :, 2:3], F.Exp, scale=-0.5)
            # one Newton step: rn = rn0*(1.5 - 0.5*m*rn0^2)
            nt_t = W("nt", 1); nt = WV(nt_t, 1)
            V.tensor_mul(nt[:], rn[:], rn[:])
            V.tensor_mul(nt[:], nt[:], lnin[:, :, 2:3])
            V.tensor_scalar(nt[:], nt[:], -0.5, 1.5, AL.mult, AL.add)
            V.tensor_mul(rn[:], rn[:], nt[:])
            ydn_t = W("ydn", 2); ydn = WV(ydn_t, 2)
            for c in range(2):
                V.tensor_mul(ydn[:, :, c:c + 1], yd[:, :, c:c + 1], rn[:])
            # sigmoid = e/(1+e); sp = 1+s; wgt = y1*sg
            rpe_t = W("rpe", 2); rpe = WV(rpe_t, 2)
            V.reciprocal(rpe[:], lnin[:, :, 0:2])
            sg_t = W("sg", 2); sg = WV(sg_t, 2)
            V.tensor_mul(sg[:], e[:], rpe[:])
            sp_t = W("sp", 2); sp = WV(sp_t, 2)
            V.tensor_scalar(sp[:], s_b[:], 1.0, None, AL.add)
            wg_t = W("wg", 2); wg = WV(wg_t, 2)
            V.tensor_mul(wg[:], sg[:], sl(R_Y1, 2))
            # J columns: Jj = sp*dy1_j + wg*p3_j
            j0_t = W("j0", 2); j0 = WV(j0_t, 2)
            j1_t = W("j1", 2); j1 = WV(j1_t, 2)
            tmp_t = W("tmp", 2); tmp = WV(tmp_t, 2)
            for jt, rowo, dst in ((0, R_DY10, j0), (1, R_DY11, j1)):
                p3o = R_P30 if jt == 0 else R_P31
                V.tensor_mul(dst[:], sp[:], sl(rowo, 2))
                V.tensor_mul(tmp[:], wg[:], sl(p3o, 2))
                V.tensor_add(dst[:], dst[:], tmp[:])
            det_t = W("det", 1); det = WV(det_t, 1)
            V.tensor_mul(det[:], j0[:, :, 0:1], j1[:, :, 1:2])
            V.tensor_mul(tmp[:, :, 0:1], j1[:, :, 0:1], j0[:, :, 1:2])
            V.tensor_sub(det[:], det[:], tmp[:, :, 0:1])
            V.reciprocal(det[:], det[:])
            xh_t = W("xh", 2); xh = WV(xh_t, 2)
            # u0 = J11*ydn0 - J01*ydn1 ; u1 = J00*ydn1 - J10*ydn0
            V.tensor_mul(xh[:, :, 0:1], j1[:, :, 1:2], ydn[:, :, 0:1])
            V.tensor_mul(tmp[:, :, 0:1], j1[:, :, 0:1], ydn[:, :, 1:2])
            V.tensor_sub(xh[:, :, 0:1], xh[:, :, 0:1], tmp[:, :, 0:1])
            V.tensor_mul(xh[:, :, 1:2], j0[:, :, 0:1], ydn[:, :, 1:2])
            V.tensor_mul(tmp[:, :, 0:1], j0[:, :, 1:2], ydn[:, :, 0:1])
            V.tensor_sub(xh[:, :, 1:2], xh[:, :, 1:2], tmp[:, :, 0:1])
            for c in range(2):
                V.tensor_mul(xh[:, :, c:c + 1], xh[:, :, c:c + 1], det[:])
            # vel scalar
            lv2_t = W("lv2", 2); lv2 = WV(lv2_t, 2)
            for c in range(2):
                V.tensor_add(lv2[:, :, c:c + 1], sl(R_X + c, 1), sl(R_LV, 1))
            ev_t = W("ev", 2); ev = WV(ev_t, 2)
            ACT(ev_t[:], lv2_t[:], F.Exp)
            V.tensor_scalar(ev_t[:], ev_t[:], 1e-12, None, AL.add)
            o_t = W("out", 2)
            V.tensor_mul(o_t[:], ev_t[:], xh_t[:])
            nc.sync.dma_start(OUTR[st], o_t[:].rearrange("p (g d) -> p g d", d=2))
            if debug and st == 0:
                nc.sync.dma_start(dbg_out("btile", [128, NG * NROW])[:], btile[:])
                nc.sync.dma_start(dbg_out("xh", [128, NG * 2])[:], xh_t[:])
                nc.sync.dma_start(dbg_out("ev", [128, NG * 2])[:], ev_t[:])
                nc.sync.dma_start(dbg_out("ydn", [128, NG * 2])[:], ydn_t[:])
                nc.sync.dma_start(dbg_out("j0", [128, NG * 2])[:], j0_t[:])
                nc.sync.dma_start(dbg_out("j1", [128, NG * 2])[:], j1_t[:])

        # ---- main loop ----
        for st in range(N_ST):
            packs, eqps, y1ts = [], [], []
            btile = pool.tile([128, NG * NROW], DT.float32, tag="btile", bufs=2)
            for tl in range(ST_TILES):
                t = st * ST_TILES + tl
                pack = pool.tile([NROW, TB], DT.float32, tag=f"pack{tl}", bufs=2)
                eqp = pool.tile([2, TB], DT.float32, tag=f"eqp{tl}", bufs=2)
                y1t = h_block_A(t, pack, eqp)
                packs.append(pack); eqps.append(eqp); y1ts.append(y1t)
            for tl in range(ST_TILES):
                ACT(eqps[tl][:], eqps[tl][:], F.Ln)  # s = ln(1+e), in place
            for tl in range(ST_TILES):
                h_block_C(st * ST_TILES + tl, tl, packs[tl], eqps[tl], y1ts[tl], btile)
            b_block_D(st, btile)

    fix_sync_waits(nc)
    return nc


def run_cores(x_full, derived, alphas, repeat=1, debug=False, modes=None):
    import time as _time
    nc = build_program(alphas, debug=debug, modes=modes)
    in_maps = []
    for c in range(N_CORES):
        m = {"x": np.ascontiguousarray(x_full[c * N_CORE:(c + 1) * N_CORE])}
        m.update(derived)
        in_maps.append(m)
    times = []
    results = None
    for r in range(repeat):
        t0 = _time.time()
        res = run_bass_kernel_spmd(nc, in_maps, list(range(N_CORES)))
        times.append(_time.time() - t0)
        results = res.results
    out = np.concatenate([results[c]["xd"] for c in range(N_CORES)], axis=0)
    if debug:
        return out, times, results[0]
    return out, times


def kernel(**inputs):
    x = np.asarray(inputs["x"], np.float32)
    derived, alphas = _host_prep(inputs)
    out, _ = run_cores(x, derived, alphas)
    return out



# revision 5
# speedup vs baseline: 8.3428x; 8.3428x over previous
"""Trainium2 Bass kernel for nn_NaturalGradientDescentVelNet.

Data-parallel over 8 NeuronCores: each core processes N/8 = 16384 points.
Per core, points are processed in 4 "super-tiles" of 8x512-point tiles.

Per tile (H-phase, hidden-dim-on-partitions layout [H, 512]):
  block A: taskmap forward (tanh MLP + elu MLP) + Jacobian tangent
           propagation (2 tangents, negated-sign trick), all ACT funcs from
           the exp_and_others table set.
  block B: softplus via ln(1+e^q3)  (natural_log_exp set -- one table
           switch per super-tile).
  block C: y = (1+s)*y1 - origin, vv net (PReLU MLP), vs net (leaky MLP),
           PE-transposes of 19 packed per-point scalars into a
           points-on-partitions B-layout tile.
  block D (per super-tile, B-layout [128, 32 groups x 19]): all per-point
           math -- sigmoid, softplus consumers, yd projection, normalize
           (ln/exp rsqrt + Newton), 2x2 adjugate inverse, vel scalar exp.

Matmul dtype per net (PE cost: f32 = 4 cyc/row, f32r = 1 cyc/row):
  tm1 fwd f32; tm2 fwd f32r; tangents f32r; vv_w1 f32r; vv_w2/w3 f32;
  vs f32r.  Host-simulated end-to-end scale-relative error ~9e-4.
"""
import numpy as np
import concourse.bass as bass
import concourse.mybir as mybir
import concourse.tile as tile
from concourse.bass_utils import run_bass_kernel_spmd

F = mybir.ActivationFunctionType
DT = mybir.dt
AL = mybir.AluOpType

N_CORES = 8
N_TOTAL = 131072
N_CORE = N_TOTAL // N_CORES       # 16384
TB = 512                          # points per tile
N_TILES = N_CORE // TB            # 32
ST_TILES = 8                      # tiles per super-tile
N_ST = N_TILES // ST_TILES        # 4
NG = ST_TILES * 4                 # 32 groups of 128 points per super-tile
NROW = 19                         # packed per-point scalars

# pack row offsets
R_X, R_Y, R_Y1, R_Q3, R_DOTY, R_LV = 0, 2, 4, 6, 8, 10
R_DY10, R_DY11, R_P30, R_P31 = 11, 13, 15, 17

DT_TM1 = "f32"    # tm1 forward
DT_TM2 = "f32"    # tm2 forward
DT_TG = "f32"     # tangents
DT_VV1 = "f32"    # vv layer 1
DT_VV23 = "f32"   # vv layers 2,3
DT_VS = "f32"     # vs net


def _f32r(dt_key):
    return DT.float32r if dt_key == "f32r" else DT.float32


def fix_sync_waits(nc, limit=1):
    """Hoist excess sem waits onto same-engine NoOps (walrus codegen limit)."""
    for fn in nc.m.functions:
        for bb in fn.blocks:
            insts = bb.instructions
            idx = 0
            while idx < len(insts):
                inst = insts[idx]
                si = inst.sync_info
                if si is not None and len(si.on_wait) > limit:
                    extra = list(si.on_wait[limit:])
                    del si.on_wait[limit:]
                    for k, w in enumerate(extra):
                        nop = mybir.InstNoOp(
                            name=f"{inst.name}-wnop{k}",
                            engine=inst.engine,
                            sync_info=mybir.SyncInfo(on_wait=[w], on_update=[]),
                        )
                        try:
                            nc.register_instruction(nop, overwrite=True)
                        except Exception:
                            pass
                        insts.insert(idx, nop)
                        idx += 1
                idx += 1


def _host_prep(inp):
    """Derived host-side constants. Returns dict of extra DRAM arrays + alphas."""
    f = {k: np.asarray(v, np.float32) for k, v in inp.items()}
    d = {}
    col = lambda a: np.ascontiguousarray(np.asarray(a, np.float32).reshape(-1, 1))
    # biases as [H,1]
    d["b1"] = col(f["tm1_b1"]); d["b2"] = col(f["tm1_b2"]); d["b3"] = col(f["tm1_b3"])
    c1 = f["tm2_b1"]
    c2p = f["tm2_b2"] - f["tm2_w2"].sum(0)
    c3p = f["tm2_b3"] - f["tm2_w3"].sum(0)
    d["c1"] = col(c1); d["nc1"] = col(-c1)
    d["c2p"] = col(c2p); d["nc2p"] = col(-c2p)
    d["c3p"] = col(c3p)
    d["vb1"] = col(f["vv_b1"]); d["vb2"] = col(f["vv_b2"]); d["vb3"] = col(f["vv_b3"])
    d["sb1"] = col(f["vs_b1"]); d["sb2"] = col(f["vs_b2"]); d["sb3"] = col(f["vs_b3"])
    # tangent seed columns (dh1'_j = u1*W1[j] - W1[j] = -(1-h1^2)W1[j])
    d["w1p0"] = col(f["tm1_w1"][0]); d["w1n0"] = col(-f["tm1_w1"][0])
    d["w1p1"] = col(f["tm1_w1"][1]); d["w1n1"] = col(-f["tm1_w1"][1])
    d["e0"] = col(np.array([1.0, 0.0])); d["e1c"] = col(np.array([0.0, 1.0]))
    d["eye"] = np.eye(NROW, dtype=np.float32)
    # origin_y = taskmap(0) in float64
    g = {k: np.asarray(v, np.float64) for k, v in inp.items()}
    z = np.zeros((1, 2))
    h = np.tanh(z @ g["tm1_w1"] + g["tm1_b1"])
    h = np.tanh(h @ g["tm1_w2"] + g["tm1_b2"])
    y1 = h @ g["tm1_w3"] + g["tm1_b3"] + z
    q = y1 @ g["tm2_w1"] + g["tm2_b1"]; gq = np.where(q > 0, q, np.expm1(q))
    q = gq @ g["tm2_w2"] + g["tm2_b2"]; gq = np.where(q > 0, q, np.expm1(q))
    q = gq @ g["tm2_w3"] + g["tm2_b3"]
    s = np.log1p(np.exp(-np.abs(q))) + np.maximum(q, 0)
    origin = (s * y1 + y1)[0]
    d["oy"] = col(origin)
    alphas = (float(f["vv_a1"][0]), float(f["vv_a2"][0]))
    # weights passed through as-is
    for k in ["tm1_w1", "tm1_w2", "tm1_w3", "tm2_w1", "tm2_w2", "tm2_w3",
              "vv_w1", "vv_w2", "vv_w3", "vs_w1", "vs_w2", "vs_w3"]:
        d[k] = f[k]
    return d, alphas


def build_program(alphas, debug=False, modes=None):
    """Build the SPMD Bass program (same for all cores)."""
    a1, a2 = alphas
    m = {"tm1": DT_TM1, "tm2": DT_TM2, "tg": DT_TG, "vv1": DT_VV1,
         "vv23": DT_VV23, "vs": DT_VS}
    if modes:
        m.update(modes)
    assert m["vv23"] == "f32", "f32r vv23 chunks not wired"
    nc = bass.Bass()
    dbg = {}
    def dbg_out(name, shape):
        if name not in dbg:
            dbg[name] = nc.declare_dram_parameter("dbg_" + name, list(shape), DT.float32, isOutput=True)
        return dbg[name]

    x_ext = nc.declare_dram_parameter("x", [N_CORE, 2], DT.float32, isOutput=False)
    out_ext = nc.declare_dram_parameter("xd", [N_CORE, 2], DT.float32, isOutput=True)

    shapes = {
        "tm1_w1": [2, 100], "tm1_w2": [100, 100], "tm1_w3": [100, 2],
        "tm2_w1": [2, 100], "tm2_w2": [100, 100], "tm2_w3": [100, 2],
        "vv_w1": [2, 300], "vv_w2": [300, 300], "vv_w3": [300, 2],
        "vs_w1": [2, 100], "vs_w2": [100, 100], "vs_w3": [100, 1],
        "b1": [100, 1], "b2": [100, 1], "b3": [2, 1],
        "c1": [100, 1], "nc1": [100, 1], "c2p": [100, 1], "nc2p": [100, 1],
        "c3p": [2, 1],
        "vb1": [300, 1], "vb2": [300, 1], "vb3": [2, 1],
        "sb1": [100, 1], "sb2": [100, 1], "sb3": [1, 1],
        "w1p0": [100, 1], "w1n0": [100, 1], "w1p1": [100, 1], "w1n1": [100, 1],
        "e0": [2, 1], "e1c": [2, 1], "oy": [2, 1], "eye": [NROW, NROW],
    }
    ext = {k: nc.declare_dram_parameter(k, v, DT.float32, isOutput=False)
           for k, v in shapes.items()}

    XR = x_ext.rearrange("(t n) d -> t d n", n=TB)             # [32, 2, 512]
    OUTR = out_ext.rearrange("(s g p) d -> s p g d", g=NG, p=128)  # [4, 128, 32, 2]

    VCH = [(0, 128), (128, 128), (256, 44)]  # K/M chunks of 300

    from contextlib import ExitStack
    with tile.TileContext(nc) as tc, ExitStack() as es:
        cpool = es.enter_context(tc.tile_pool(name="const", bufs=1))
        pool = es.enter_context(tc.tile_pool(name="work", bufs=1))
        pp = es.enter_context(tc.tile_pool(name="ps", bufs=1, space="PSUM"))

        # ---- constants into SBUF (chunk-only tensors excluded) ----
        CHUNK_ONLY = {"vv_w2", "vv_w3", "vb1", "vb2"}
        ct = {}
        for k, shp in shapes.items():
            if k in CHUNK_ONLY:
                continue
            t = cpool.tile(shp, DT.float32, tag="c_" + k)
            nc.sync.dma_start(t[:], ext[k][:])
            ct[k] = t
        # chunked vv weights / biases
        vv_w2f = []
        vv_w3f = []
        a_vb1, a_vb2 = [], []
        for (o, sz) in VCH:
            t = cpool.tile([sz, 300], DT.float32, tag=f"c_vvw2_{o}")
            nc.sync.dma_start(t[:], ext["vv_w2"][o:o + sz, :])
            vv_w2f.append(t)
            t = cpool.tile([sz, 2], DT.float32, tag=f"c_vvw3_{o}")
            nc.sync.dma_start(t[:], ext["vv_w3"][o:o + sz, :])
            vv_w3f.append(t)
            t = cpool.tile([sz, 1], DT.float32, tag=f"c_vb1_{o}")
            nc.sync.dma_start(t[:], ext["vb1"][o:o + sz, :])
            a_vb1.append(t)
            t = cpool.tile([sz, 1], DT.float32, tag=f"c_vb2_{o}")
            nc.sync.dma_start(t[:], ext["vb2"][o:o + sz, :])
            a_vb2.append(t)

        # f32r-rounded weight copies (producer must round for f32r matmuls)
        def r_copy(name, src):
            t = cpool.tile(list(src.shape), DT.float32r, tag="cr_" + name,
                           name="cr_" + name)
            nc.vector.tensor_copy(t[:], src[:])
            return t
        rcache = {}
        def wsel(name, mode):
            if mode == "f32":
                return ct[name]
            if name not in rcache:
                rcache[name] = r_copy(name, ct[name])
            return rcache[name]
        w_tm1w2_tg = wsel("tm1_w2", m["tg"])
        w_tm1w3_tg = wsel("tm1_w3", m["tg"])
        w_tm2w1_f = wsel("tm2_w1", m["tm2"])
        w_tm2w2_f = wsel("tm2_w2", m["tm2"])
        w_tm2w3_f = wsel("tm2_w3", m["tm2"])
        w_tm2w1_tg = wsel("tm2_w1", m["tg"])
        w_tm2w2_tg = wsel("tm2_w2", m["tg"])
        w_tm2w3_tg = wsel("tm2_w3", m["tg"])
        w_vv1 = wsel("vv_w1", m["vv1"])
        w_vs1 = wsel("vs_w1", m["vs"])
        w_vs2 = wsel("vs_w2", m["vs"])
        w_vs3 = wsel("vs_w3", m["vs"])
        DTG = _f32r(m["tg"]); DTM2 = _f32r(m["tm2"])
        DVV1 = _f32r(m["vv1"]); DVS = _f32r(m["vs"])

        MM = nc.tensor.matmul
        ACT = nc.scalar.activation
        V = nc.vector

        def h_block_A(t, pack, eqp):
            """taskmap fwd + tangents for tile t. Writes pack rows and
            eqp = 1 + exp(q3). Returns f32r dy1 tiles."""
            nc.sync.dma_start(pack[R_X:R_X + 2, :], XR[t])
            # tm1 forward (f32)
            ps = pp.tile([128, TB], DT.float32, tag="ps", bufs=6)
            MM(ps[0:100, :], ct["tm1_w1"][:], pack[R_X:R_X + 2, :], start=True, stop=True)
            h1 = pool.tile([100, TB], DT.float32, tag="h1", bufs=2)
            ACT(h1[:], ps[0:100, :], F.Tanh, bias=ct["b1"][:])
            u1 = pool.tile([100, TB], DT.float32, tag="u1", bufs=1)
            ACT(u1[:], h1[:], F.Square)
            ps2 = pp.tile([128, TB], DT.float32, tag="ps", bufs=6)
            MM(ps2[0:100, :], ct["tm1_w2"][:], h1[:], start=True, stop=True)
            h2 = pool.tile([100, TB], DT.float32, tag="h2", bufs=2)
            ACT(h2[:], ps2[0:100, :], F.Tanh, bias=ct["b2"][:])
            u2 = pool.tile([100, TB], DT.float32, tag="u2", bufs=1)
            ACT(u2[:], h2[:], F.Square)
            ps3 = pp.tile([128, TB], DT.float32, tag="ps", bufs=6)
            MM(ps3[0:2, :], ct["tm1_w3"][:], h2[:], start=True, stop=True)
            y1t = pool.tile([2, TB], DT.float32, tag=f"y1t{t % 8}", bufs=2)
            V.tensor_scalar(y1t[:], ps3[0:2, :], ct["b3"][:], None, AL.add)
            V.tensor_add(y1t[:], y1t[:], pack[R_X:R_X + 2, :])
            nc.sync.dma_start(pack[R_Y1:R_Y1 + 2, :], y1t[:])
            if m["tm2"] == "f32r":
                y1r = pool.tile([2, TB], DT.float32r, tag="y1r", bufs=1)
                V.tensor_copy(y1r[:], y1t[:])
            else:
                y1r = y1t

            # tm1 tangents (negated): dh1'_j = u1*w1p_j - w1p_j
            dy1r = []
            dh2r = []
            for j, (wp, wn) in enumerate([("w1p0", "w1n0"), ("w1p1", "w1n1")]):
                dh1 = pool.tile([100, TB], DTG, tag=f"dh1_{j}", bufs=1)
                V.tensor_scalar(dh1[:], u1[:], ct[wp][:], ct[wn][:], AL.mult, AL.add)
                psd = pp.tile([128, TB], DT.float32, tag="ps", bufs=6)
                MM(psd[0:100, :], w_tm1w2_tg[:], dh1[:], start=True, stop=True)
                dh2 = pool.tile([100, TB], DTG, tag=f"dh2_{j}", bufs=1)
                # dh2' = (u2-1)*psd = (1-h2^2)*(true tangent)
                V.scalar_tensor_tensor(dh2[:], u2[:], 1.0, psd[0:100, :], AL.subtract, AL.mult)
                dh2r.append(dh2)
            for j, (dh2, ec, rowo) in enumerate([(dh2r[0], "e0", R_DY10), (dh2r[1], "e1c", R_DY11)]):
                psd = pp.tile([128, TB], DT.float32, tag="ps", bufs=6)
                MM(psd[0:2, :], w_tm1w3_tg[:], dh2[:], start=True, stop=True)
                # dh2' double-negated back to true sign, so dy1 = psd + e_j
                dyt = pool.tile([2, TB], DT.float32, tag=f"dyt_{j}", bufs=1)
                V.tensor_scalar(dyt[:], psd[0:2, :], ct[ec][:], None, AL.add)
                nc.sync.dma_start(pack[rowo:rowo + 2, :], dyt[:])
                if m["tg"] == "f32r":
                    dr = pool.tile([2, TB], DT.float32r, tag=f"dy1r_{j}", bufs=1)
                    V.tensor_copy(dr[:], dyt[:])
                else:
                    dr = dyt
                dy1r.append(dr)

            # tm2 forward (f32r) with elu
            def elu_layer(rhs, cb, ncb, tagp):
                psq = pp.tile([128, TB], DT.float32, tag="ps", bufs=6)
                wq = w_tm2w1_f if tagp == "1" else w_tm2w2_f
                MM(psq[0:100, :], wq[:], rhs[:], start=True, stop=True)
                rn = pool.tile([100, TB], DT.float32, tag="rn" + tagp, bufs=1)
                ACT(rn[:], psq[0:100, :], F.Relu, bias=ct[ncb][:], scale=-1.0)
                e = pool.tile([100, TB], DT.float32, tag="e" + tagp, bufs=1)
                ACT(e[:], rn[:], F.Exp, scale=-1.0)
                gh = pool.tile([100, TB], DT.float32, tag="gh" + tagp, bufs=1)
                ACT(gh[:], psq[0:100, :], F.Relu, bias=ct[cb][:])
                gu = pool.tile([100, TB], DTM2, tag="gu" + tagp, bufs=1)
                V.tensor_add(gu[:], gh[:], e[:])
                return e, gu
            e1t, g1u = elu_layer(y1r, "c1", "nc1", "1")
            e2t, g2u = elu_layer(g1u, "c2p", "nc2p", "2")
            psq3 = pp.tile([128, TB], DT.float32, tag="ps", bufs=6)
            MM(psq3[0:2, :], w_tm2w3_f[:], g2u[:], start=True, stop=True)
            q3t = pool.tile([2, TB], DT.float32, tag="q3t", bufs=1)
            V.tensor_scalar(q3t[:], psq3[0:2, :], ct["c3p"][:], None, AL.add)
            nc.sync.dma_start(pack[R_Q3:R_Q3 + 2, :], q3t[:])
            # eqp = exp(q3) + 1  (ln input for softplus)
            ACT(eqp[:], psq3[0:2, :], F.Exp, bias=ct["c3p"][:])
            V.tensor_scalar(eqp[:], eqp[:], 1.0, None, AL.add)

            # tm2 tangents (negated stream)
            for j, (dr, rowo) in enumerate([(dy1r[0], R_P30), (dy1r[1], R_P31)]):
                # note: matmul wants f32r rhs; dr is true-sign f32r
                psg = pp.tile([128, TB], DT.float32, tag="ps", bufs=6)
                MM(psg[0:100, :], w_tm2w1_tg[:], dr[:], start=True, stop=True)
                dg1 = pool.tile([100, TB], DTG, tag=f"dg1_{j}", bufs=1)
                V.tensor_mul(dg1[:], e1t[:], psg[0:100, :])
                psg2 = pp.tile([128, TB], DT.float32, tag="ps", bufs=6)
                MM(psg2[0:100, :], w_tm2w2_tg[:], dg1[:], start=True, stop=True)
                dg2 = pool.tile([100, TB], DTG, tag=f"dg2_{j}", bufs=1)
                V.tensor_mul(dg2[:], e2t[:], psg2[0:100, :])
                psg3 = pp.tile([128, TB], DT.float32, tag="ps", bufs=6)
                MM(psg3[0:2, :], w_tm2w3_tg[:], dg2[:], start=True, stop=True)
                p3t = pool.tile([2, TB], DT.float32, tag=f"p3t_{j}", bufs=1)
                V.tensor_copy(p3t[:], psg3[0:2, :])
                nc.sync.dma_start(pack[rowo:rowo + 2, :], p3t[:])
            if debug and t == 0:
                nc.sync.dma_start(dbg_out("h1", [100, TB])[:], h1[:])
                nc.sync.dma_start(dbg_out("e1", [100, TB])[:], e1t[:])
                nc.sync.dma_start(dbg_out("g1u", [100, TB])[:], g1u[:].bitcast(DT.float32))
                nc.sync.dma_start(dbg_out("y1t", [2, TB])[:], y1t[:])
                nc.sync.dma_start(dbg_out("q3t", [2, TB])[:], q3t[:])
            return y1t

        def h_block_C(t, tl, pack, s_t, y1t, btile):
            """y, vv net, vs net, transpose into btile cols for tile t."""
            # y = (s+1)*y1 - origin
            ypre = pool.tile([2, TB], DT.float32, tag="ypre", bufs=1)
            V.scalar_tensor_tensor(ypre[:], s_t[:], 1.0, y1t[:], AL.add, AL.mult)
            yt = pool.tile([2, TB], DT.float32, tag="yt", bufs=1)
            V.tensor_scalar(yt[:], ypre[:], ct["oy"][:], None, AL.subtract)
            nc.sync.dma_start(pack[R_Y:R_Y + 2, :], yt[:])
            if m["vv1"] == "f32r":
                y_r = pool.tile([2, TB], DT.float32r, tag="y_r", bufs=1)
                V.tensor_copy(y_r[:], yt[:])
            else:
                y_r = yt
            if m["vs"] == "f32r":
                x_r = pool.tile([2, TB], DT.float32r, tag="x_r", bufs=1)
                V.tensor_copy(x_r[:], pack[R_X:R_X + 2, :])
            else:
                x_r = None  # use pack slice directly

            # vv layer 1 (f32r) + prelu
            a1t = []
            for ci, (o, sz) in enumerate(VCH):
                psv = pp.tile([128, TB], DT.float32, tag="ps", bufs=6)
                MM(psv[0:sz, :], w_vv1[:, o:o + sz], y_r[:], start=True, stop=True)
                at = pool.tile([sz, TB], DT.float32, tag=f"a1_{ci}", bufs=1)
                ACT(at[:], psv[0:sz, :], F.Prelu, bias=a_vb1[ci][:], alpha=a1)
                a1t.append(at)
            # vv layer 2 (f32) + prelu
            a2t = []
            for ci, (o, sz) in enumerate(VCH):
                psv = pp.tile([128, TB], DT.float32, tag="ps", bufs=6)
                for ki, (ko, ksz) in enumerate(VCH):
                    MM(psv[0:sz, :], vv_w2f[ki][:, o:o + sz], a1t[ki][:],
                       start=(ki == 0), stop=(ki == 2))
                at = pool.tile([sz, TB], DT.float32, tag=f"a2_{ci}", bufs=1)
                ACT(at[:], psv[0:sz, :], F.Prelu, bias=a_vb2[ci][:], alpha=a2)
                a2t.append(at)
            # vv layer 3 (f32)
            psd = pp.tile([128, TB], DT.float32, tag="ps", bufs=6)
            for ki, (ko, ksz) in enumerate(VCH):
                MM(psd[0:2, :], vv_w3f[ki][:], a2t[ki][:], start=(ki == 0), stop=(ki == 2))
            dotyt = pool.tile([2, TB], DT.float32, tag="dotyt", bufs=1)
            V.tensor_scalar(dotyt[:], psd[0:2, :], ct["vb3"][:], None, AL.add)
            nc.sync.dma_start(pack[R_DOTY:R_DOTY + 2, :], dotyt[:])

            # vs net (f32r)
            psr = pp.tile([128, TB], DT.float32, tag="ps", bufs=6)
            xin = x_r[:] if x_r is not None else pack[R_X:R_X + 2, :]
            MM(psr[0:100, :], w_vs1[:], xin, start=True, stop=True)
            l1 = pool.tile([100, TB], DVS, tag="l1", bufs=1)
            ACT(l1[:], psr[0:100, :], F.Prelu, bias=ct["sb1"][:], alpha=0.01)
            psr2 = pp.tile([128, TB], DT.float32, tag="ps", bufs=6)
            MM(psr2[0:100, :], w_vs2[:], l1[:], start=True, stop=True)
            l2 = pool.tile([100, TB], DVS, tag="l2", bufs=1)
            ACT(l2[:], psr2[0:100, :], F.Prelu, bias=ct["sb2"][:], alpha=0.01)
            psr3 = pp.tile([128, TB], DT.float32, tag="ps", bufs=6)
            MM(psr3[0:1, :], w_vs3[:], l2[:], start=True, stop=True)
            lvt = pool.tile([1, TB], DT.float32, tag="lvt", bufs=1)
            V.tensor_scalar(lvt[:], psr3[0:1, :], ct["sb3"][:], None, AL.add)
            nc.sync.dma_start(pack[R_LV:R_LV + 1, :], lvt[:])
            if debug and t == 0:
                nc.sync.dma_start(dbg_out("s0", [2, TB])[:], s_t[:])
                nc.sync.dma_start(dbg_out("yt", [2, TB])[:], yt[:])
                nc.sync.dma_start(dbg_out("a1c0", [128, TB])[:], a1t[0][:])
                nc.sync.dma_start(dbg_out("l1", [100, TB])[:], l1[:].bitcast(DT.float32))
                nc.sync.dma_start(dbg_out("lvt", [1, TB])[:], lvt[:])
                nc.sync.dma_start(dbg_out("pack0", [NROW, TB])[:], pack[:])

            # transpose pack -> btile  (4 chunks of 128 points)
            psT = pp.tile([128, 4 * NROW], DT.float32, tag="psT", bufs=2)
            for u in range(4):
                nc.tensor.transpose(psT[:, u * NROW:(u + 1) * NROW],
                                    pack[:, u * 128:(u + 1) * 128], ct["eye"][:])
            V.tensor_copy(btile[:, tl * 4 * NROW:(tl + 1) * 4 * NROW], psT[:])

        def b_block_D(st, btile):
            """per-point math for one super-tile; writes output DMA."""
            BV = btile[:].rearrange("p (g r) -> p g r", r=NROW)
            sl = lambda k, w: BV[:, :, k:k + w]

            def W(tag, w, b=1):
                return pool.tile([128, NG * w], DT.float32, tag="bw_" + tag,
                                 bufs=b, name=f"bw_{tag}_{st}")
            def WV(t, w):
                return t[:].rearrange("p (g r) -> p g r", r=w)

            e_t = W("e", 2); e = WV(e_t, 2)
            ACT(e_t[:], sl(R_Q3, 2), F.Exp)
            lnin_t = W("lnin", 3); lnin = WV(lnin_t, 3)
            V.tensor_scalar(lnin[:, :, 0:2], e[:], 1.0, None, AL.add)
            # yd path
            p2_t = W("p2", 2); p2 = WV(p2_t, 2)
            V.tensor_mul(p2[:], sl(R_DOTY, 2), sl(R_Y, 2))
            ls_t = W("ls", 1); ls = WV(ls_t, 1)
            V.tensor_add(ls[:], p2[:, :, 0:1], p2[:, :, 1:2])
            V.tensor_mul(p2[:], sl(R_Y, 2), sl(R_Y, 2))
            vy_t = W("vy", 1); vy = WV(vy_t, 1)
            V.tensor_add(vy[:], p2[:, :, 0:1], p2[:, :, 1:2])
            rv_t = W("rv", 1); rv = WV(rv_t, 1)
            V.scalar_tensor_tensor(rv[:], vy[:], 1e-4, ls[:], AL.mult, AL.add)
            V.tensor_scalar(rv[:], rv[:], 0.0, None, AL.max)
            den_t = W("den", 1); den = WV(den_t, 1)
            V.tensor_scalar(den[:], vy[:], 1e-12, None, AL.add)
            V.reciprocal(den[:], den[:])
            V.tensor_mul(rv[:], rv[:], den[:])          # coef
            yd_t = W("yd", 2); yd = WV(yd_t, 2)
            for c in range(2):
                V.tensor_mul(yd[:, :, c:c + 1], rv[:], sl(R_Y + c, 1))
                V.tensor_sub(yd[:, :, c:c + 1], sl(R_DOTY + c, 1), yd[:, :, c:c + 1])
            V.tensor_mul(p2[:], yd[:], yd[:])
            V.tensor_add(lnin[:, :, 2:3], p2[:, :, 0:1], p2[:, :, 1:2])
            V.tensor_scalar(lnin[:, :, 2:3], lnin[:, :, 2:3], 1e-24, None, AL.max)
            ln_t = W("ln", 3); lnv = WV(ln_t, 3)
            ACT(ln_t[:], lnin_t[:], F.Ln)
            s_b = lnv[:, :, 0:2]
            rn_t = W("rn", 1); rn = WV(rn_t, 1)
            ACT(rn_t[:], lnv[:, :, 2:3], F.Exp, scale=-0.5)
            # one Newton step: rn = rn0*(1.5 - 0.5*m*rn0^2)
            nt_t = W("nt", 1); nt = WV(nt_t, 1)
            V.tensor_mul(nt[:], rn[:], rn[:])
            V.tensor_mul(nt[:], nt[:], lnin[:, :, 2:3])
            V.tensor_scalar(nt[:], nt[:], -0.5, 1.5, AL.mult, AL.add)
            V.tensor_mul(rn[:], rn[:], nt[:])
            ydn_t = W("ydn", 2); ydn = WV(ydn_t, 2)
            for c in range(2):
                V.tensor_mul(ydn[:, :, c:c + 1], yd[:, :, c:c + 1], rn[:])
            # sigmoid = e/(1+e); sp = 1+s; wgt = y1*sg
            rpe_t = W("rpe", 2); rpe = WV(rpe_t, 2)
            V.reciprocal(rpe[:], lnin[:, :, 0:2])
            sg_t = W("sg", 2); sg = WV(sg_t, 2)
            V.tensor_mul(sg[:], e[:], rpe[:])
            sp_t = W("sp", 2); sp = WV(sp_t, 2)
            V.tensor_scalar(sp[:], s_b[:], 1.0, None, AL.add)
            wg_t = W("wg", 2); wg = WV(wg_t, 2)
            V.tensor_mul(wg[:], sg[:], sl(R_Y1, 2))
            # J columns: Jj = sp*dy1_j + wg*p3_j
            j0_t = W("j0", 2); j0 = WV(j0_t, 2)
            j1_t = W("j1", 2); j1 = WV(j1_t, 2)
            tmp_t = W("tmp", 2); tmp = WV(tmp_t, 2)
            for jt, rowo, dst in ((0, R_DY10, j0), (1, R_DY11, j1)):
                p3o = R_P30 if jt == 0 else R_P31
                V.tensor_mul(dst[:], sp[:], sl(rowo, 2))
                V.tensor_mul(tmp[:], wg[:], sl(p3o, 2))
                V.tensor_add(dst[:], dst[:], tmp[:])
            det_t = W("det", 1); det = WV(det_t, 1)
            V.tensor_mul(det[:], j0[:, :, 0:1], j1[:, :, 1:2])
            V.tensor_mul(tmp[:, :, 0:1], j1[:, :, 0:1], j0[:, :, 1:2])
            V.tensor_sub(det[:], det[:], tmp[:, :, 0:1])
            V.reciprocal(det[:], det[:])
            xh_t = W("xh", 2); xh = WV(xh_t, 2)
            # u0 = J11*ydn0 - J01*ydn1 ; u1 = J00*ydn1 - J10*ydn0
            V.tensor_mul(xh[:, :, 0:1], j1[:, :, 1:2], ydn[:, :, 0:1])
            V.tensor_mul(tmp[:, :, 0:1], j1[:, :, 0:1], ydn[:, :, 1:2])
            V.tensor_sub(xh[:, :, 0:1], xh[:, :, 0:1], tmp[:, :, 0:1])
            V.tensor_mul(xh[:, :, 1:2], j0[:, :, 0:1], ydn[:, :, 1:2])
            V.tensor_mul(tmp[:, :, 0:1], j0[:, :, 1:2], ydn[:, :, 0:1])
            V.tensor_sub(xh[:, :, 1:2], xh[:, :, 1:2], tmp[:, :, 0:1])
            for c in range(2):
                V.tensor_mul(xh[:, :, c:c + 1], xh[:, :, c:c + 1], det[:])
            # vel scalar
            lv2_t = W("lv2", 2); lv2 = WV(lv2_t, 2)
            for c in range(2):
                V.tensor_add(lv2[:, :, c:c + 1], sl(R_X + c, 1), sl(R_LV, 1))
            ev_t = W("ev", 2); ev = WV(ev_t, 2)
            ACT(ev_t[:], lv2_t[:], F.Exp)
            V.tensor_scalar(ev_t[:], ev_t[:], 1e-12, None, AL.add)
            o_t = W("out", 2)
            V.tensor_mul(o_t[:], ev_t[:], xh_t[:])
            nc.sync.dma_start(OUTR[st], o_t[:].rearrange("p (g d) -> p g d", d=2))
            if debug and st == 0:
                nc.sync.dma_start(dbg_out("btile", [128, NG * NROW])[:], btile[:])
                nc.sync.dma_start(dbg_out("xh", [128, NG * 2])[:], xh_t[:])
                nc.sync.dma_start(dbg_out("ev", [128, NG * 2])[:], ev_t[:])
                nc.sync.dma_start(dbg_out("ydn", [128, NG * 2])[:], ydn_t[:])
                nc.sync.dma_start(dbg_out("j0", [128, NG * 2])[:], j0_t[:])
                nc.sync.dma_start(dbg_out("j1", [128, NG * 2])[:], j1_t[:])

        # ---- main loop ----
        for st in range(N_ST):
            packs, eqps, y1ts = [], [], []
            btile = pool.tile([128, NG * NROW], DT.float32, tag="btile", bufs=2)
            for tl in range(ST_TILES):
                t = st * ST_TILES + tl
                pack = pool.tile([NROW, TB], DT.float32, tag=f"pack{tl}", bufs=2)
                eqp = pool.tile([2, TB], DT.float32, tag=f"eqp{tl}", bufs=2)
                y1t = h_block_A(t, pack, eqp)
                packs.append(pack); eqps.append(eqp); y1ts.append(y1t)
            for tl in range(ST_TILES):
                ACT(eqps[tl][:], eqps[tl][:], F.Ln)  # s = ln(1+e), in place
            for tl in range(ST_TILES):
                h_block_C(st * ST_TILES + tl, tl, packs[tl], eqps[tl], y1ts[tl], btile)
            b_block_D(st, btile)

    fix_sync_waits(nc)
    return nc


_RUNNER_CACHE = {}


def _build_runner(alphas, derived, debug=False, modes=None):
    """Build jit(shard_map(bass_exec)) once; weights device-resident.

    Per call only x is uploaded and xd downloaded. Output zero-buffers are
    created on-device inside the jit (our kernel writes every element of xd).
    """
    import jax
    import jax.numpy as jnp
    from jax.sharding import Mesh, PartitionSpec, NamedSharding
    from jax.experimental.shard_map import shard_map
    from concourse.bass2jax import (
        _bass_exec_p, install_neuronx_cc_hook, partition_id_tensor)

    install_neuronx_cc_hook()
    nc = build_program(alphas, debug=debug, modes=modes)
    partition_name = nc.partition_id_tensor.name if nc.partition_id_tensor else None

    in_names, out_names, out_avals = [], [], []
    for alloc in nc.m.functions[0].allocations:
        if not isinstance(alloc, mybir.MemoryLocationSet):
            continue
        name = alloc.memorylocations[0].name
        if alloc.kind == "ExternalInput":
            if name != partition_name:
                in_names.append(name)
        elif alloc.kind == "ExternalOutput":
            out_names.append(name)
            out_avals.append(jax.core.ShapedArray(
                tuple(alloc.tensor_shape), mybir.dt.np(alloc.dtype)))
    all_in = list(in_names) + list(out_names)
    if partition_name is not None:
        all_in.append(partition_name)

    def _body(*args):
        operands = list(args)
        if partition_name is not None:
            operands.append(partition_id_tensor())
        return tuple(_bass_exec_p.bind(
            *operands,
            out_avals=tuple(out_avals),
            in_names=tuple(all_in),
            out_names=tuple(out_names),
            lowering_input_output_aliases=(),
            sim_require_finite=True,
            sim_require_nnan=True,
            nc=nc))

    devices = jax.devices()[:N_CORES]
    mesh = Mesh(np.asarray(devices), ("core",))
    spec = PartitionSpec("core")
    n_io = len(in_names) + len(out_names)
    fn = jax.jit(shard_map(
        _body, mesh=mesh, in_specs=(spec,) * n_io,
        out_specs=(spec,) * len(out_names), check_rep=False),
        keep_unused=True)

    # upload replicated weights once (concat 8 copies along axis 0);
    # zero output buffers also resident (backend writes PJRT result buffers,
    # operand content is unused — our kernel writes every element of xd)
    sh = NamedSharding(mesh, spec)
    resident = {}
    for nm in in_names:
        if nm == "x":
            continue
        a = np.asarray(derived[nm], np.float32)
        resident[nm] = jax.device_put(
            np.concatenate([a] * N_CORES, axis=0), sh)
    rzero = []
    for nm, aval in zip(out_names, out_avals):
        z = np.zeros((N_CORES * aval.shape[0], *aval.shape[1:]), aval.dtype)
        rzero.append(jax.device_put(z, sh))
    jax.block_until_ready(list(resident.values()) + rzero)

    def call(x_full):
        x_dev = jax.device_put(np.asarray(x_full, np.float32), sh)
        args = [x_dev if nm == "x" else resident[nm] for nm in in_names]
        outs = fn(*args, *rzero)
        return {nm: outs[i] for i, nm in enumerate(out_names)}

    # warm-up: trace + XLA/NEFF compile
    jax.block_until_ready(list(call(np.zeros((N_TOTAL, 2), np.float32)).values()))
    return call, out_names


def run_cores(x_full, derived, alphas, repeat=1, debug=False, modes=None):
    import time as _time
    key = (bool(debug), tuple(sorted(modes.items())) if modes else None)
    if key not in _RUNNER_CACHE:
        _RUNNER_CACHE[key] = _build_runner(alphas, derived, debug=debug, modes=modes)
    call, out_names = _RUNNER_CACHE[key]
    times = []
    outs = None
    for r in range(repeat):
        t0 = _time.time()
        outs = call(x_full)
        out = np.asarray(outs["xd"])
        times.append(_time.time() - t0)
    if debug:
        dbg = {nm: np.asarray(v).reshape(N_CORES, -1, v.shape[-1])[0]
               for nm, v in outs.items()}
        return out, times, dbg
    return out, times


def kernel(**inputs):
    x = np.asarray(inputs["x"], np.float32)
    derived, alphas = _host_prep(inputs)
    out, _ = run_cores(x, derived, alphas)
    return out

